# revision 1
# baseline (speedup 1.0000x reference)
"""CstLoss on Trainium2 — self-contained Bass/Tile SPMD kernel (8 NeuronCores).

Reference math (per [N=64, C=17, H=128, W=128] f32 pair output/target):
  h/w marginal means of each map -> softmax over the 128-axis -> l2
  normalize -> sim_pos = mean of matched-channel cosines, sim = sum of
  mean-over-batch all-pairs cosines, loss = -log(sim_pos/sim)/C/N.

Key algebra used here:
  * softmax denominator cancels under l2 normalization (and the reference's
    1e-8 norm clamp never binds since max(exp) = 1), so each projection only
    needs q = e / ||e||_2 with e = exp((S - max S)/W), S = raw row/col sums.
  * sum_ij dot(qo_i, qt_j) = dot(sum_i qo_i, sum_j qt_j), so the CxC pair
    matrix is never materialized: per n we only need channel sums U, V.

Sharding: data-parallel over the batch axis, 8 entries per core. Each core
returns per-map matched dots ("pos" [136]) and channel sums ("u"/"v"
[8, 256]); the host reduces those to the scalar loss (the two "all-reduce a
scalar" steps of the hint, done on host).

Per-core kernel: 136 maps per tensor = 128-map main batch in natural layout
(one full 64KB map per SBUF partition -> large contiguous DMA descriptors)
+ 8-map tail in 2D layout (h on partitions). h-projections: DVE segmented
reduces. w-projections: PE transpose-accumulate of h-slices into PSUM
(exact: transposes move data, PSUM accumulates in f32). Tail w-projections:
ones-vector matmuls + SBUF scatter DMA.
"""

import contextlib
import ctypes
import sys
import types
from contextlib import ExitStack

import numpy as np

import concourse.bacc as bacc
import concourse.tile as tile
from concourse import mybir
from concourse.bass_utils import run_bass_kernel_spmd

F32 = mybir.dt.float32
AX = mybir.AxisListType
ALU = mybir.AluOpType
ACT = mybir.ActivationFunctionType

N, C, H, W = 64, 17, 128, 128
NCORES = 8
NLOC = N // NCORES           # 8 batch entries per core
MAPS = NLOC * C              # 136 maps per tensor per core
MAIN = 128                   # maps in the main batch
TAIL = MAPS - MAIN           # 8 maps in the tail
HCHUNK = 32                  # h-rows per main sub-chunk
NCHUNKS = H // HCHUNK


def _install_ntff_hook():
    """Provide antenv.axon_hooks if the image lacks it (needed only when
    run_bass_kernel_spmd is called with trace=True; harmless otherwise)."""
    if "antenv.axon_hooks" in sys.modules:
        return
    so_path = "/opt/axon/libaxon_pjrt.so"
    hook = None
    try:
        lib = ctypes.CDLL(so_path)
        if hasattr(lib, "axon_start_nrt_profile"):
            lib.axon_start_nrt_profile.argtypes = [
                ctypes.POINTER(ctypes.c_int64),
                ctypes.c_size_t,
            ]
            lib.axon_start_nrt_profile.restype = ctypes.c_int64
            lib.axon_stop_nrt_profile.argtypes = [ctypes.c_char_p]
            lib.axon_stop_nrt_profile.restype = ctypes.c_int64

            @contextlib.contextmanager
            def _hook(output_dir, device_ids):
                import jax

                jax.devices()
                if device_ids:
                    ids = (ctypes.c_int64 * len(device_ids))(*device_ids)
                    rc = lib.axon_start_nrt_profile(ids, len(device_ids))
                else:
                    rc = lib.axon_start_nrt_profile(None, 0)
                if rc != 0:
                    raise RuntimeError(f"axon_start_nrt_profile rc={rc}")
                try:
                    yield
                finally:
                    n = lib.axon_stop_nrt_profile(str(output_dir).encode())
                    print(f"profile: {n} file(s) in {output_dir}", file=sys.stderr)

            hook = _hook
    except OSError:
        pass
    mod = types.ModuleType("antenv.axon_hooks")
    mod.get_axon_ntff_profile_hook = lambda: hook
    mod.set_axon_ntff_profile_hook = lambda h: None
    sys.modules["antenv.axon_hooks"] = mod


_install_ntff_hook()


def _normalize(nc, workp, ap, Pn, pref):
    """In place per 128-segment: e = exp((S - max S)/W); q = e/||e||_2."""
    v = ap.rearrange("p (s w) -> p s w", w=W)
    mx = workp.tile([Pn, 2], F32, tag=f"mx{Pn}", name=f"mx_{pref}")
    nc.vector.reduce_max(mx[:], v, axis=AX.X)
    nb = workp.tile([Pn, 2], F32, tag=f"nb{Pn}", name=f"nb_{pref}")
    nc.scalar.mul(nb[:], mx[:], -1.0 / W)
    ssq = workp.tile([Pn, 2], F32, tag=f"ssq{Pn}", name=f"ssq_{pref}")
    dump = workp.tile([Pn, W], F32, tag=f"dump{Pn}", name=f"dump_{pref}")
    for s in range(2):
        nc.scalar.activation(
            v[:, s, :], v[:, s, :], ACT.Exp, bias=nb[:, s : s + 1], scale=1.0 / W
        )
        # sum of squares: ACT Square + accum_out (tensor_tensor_reduce
        # wedges the device on this runtime)
        nc.scalar.activation(dump[:], v[:, s, :], ACT.Square, accum_out=ssq[:, s : s + 1])
    sq = workp.tile([Pn, 2], F32, tag=f"sq{Pn}", name=f"sq_{pref}")
    nc.scalar.sqrt(sq[:], ssq[:])
    rn = workp.tile([Pn, 2], F32, tag=f"rn{Pn}", name=f"rn_{pref}")
    nc.vector.reciprocal(rn[:], sq[:])
    for s in range(2):
        nc.vector.tensor_scalar_mul(v[:, s, :], v[:, s, :], rn[:, s : s + 1])


def _body(tc, o_d, t_d, id_d, g0_d, gt_d, on_d, pos_d, u_d, v_d):
    nc = tc.nc
    with ExitStack() as ctx:
        consts = ctx.enter_context(tc.tile_pool(name="consts", bufs=1))
        chunks = ctx.enter_context(tc.tile_pool(name="chunks", bufs=8))
        projp = ctx.enter_context(tc.tile_pool(name="projp", bufs=1))
        tailp = ctx.enter_context(tc.tile_pool(name="tailp", bufs=1))
        workp = ctx.enter_context(tc.tile_pool(name="workp", bufs=2))
        outp = ctx.enter_context(tc.tile_pool(name="outp", bufs=1))
        # PSUM: 8 distinct tiles = 8 banks, no slot rotation (slot reuse
        # with concurrent PE traffic wedges the device: NRT status 101).
        accps = ctx.enter_context(tc.tile_pool(name="accps", bufs=1, space="PSUM"))

        ident = consts.tile([128, 128], F32)
        nc.gpsimd.dma_start(ident[:], id_d)
        g0 = consts.tile([128, NLOC], F32)
        nc.gpsimd.dma_start(g0[:], g0_d)
        gt = consts.tile([TAIL, NLOC], F32)
        nc.gpsimd.dma_start(gt[:], gt_d)
        ones = consts.tile([128, 1], F32)
        nc.gpsimd.dma_start(ones[:], on_d)

        proj_o = projp.tile([128, 2 * W], F32)
        proj_t = projp.tile([128, 2 * W], F32)
        wt_o = accps.tile([128, 128], F32)
        wt_t = accps.tile([128, 128], F32)
        U = accps.tile([NLOC, 2 * W], F32)
        Vt = accps.tile([NLOC, 2 * W], F32)

        # ---- main batches: 128 maps, one full map per partition ----
        for ti, (x_d, proj, wt) in enumerate(
            ((o_d, proj_o, wt_o), (t_d, proj_t, wt_t))
        ):
            for c in range(NCHUNKS):
                chunk = chunks.tile(
                    [128, HCHUNK * W], F32, tag="chunk", name=f"chunk{ti}_{c}"
                )
                nc.sync.dma_start(
                    chunk[:], x_d[0:MAIN, c * HCHUNK : (c + 1) * HCHUNK, :]
                )
                cv = chunk.rearrange("p (h w) -> p h w", w=W)
                nc.vector.reduce_sum(
                    proj[:, c * HCHUNK : (c + 1) * HCHUNK], cv, axis=AX.X
                )
                for j in range(HCHUNK):
                    nc.tensor.matmul(
                        wt[:],
                        cv[:, j, :],
                        ident[:],
                        is_transpose=True,
                        start=(c == 0 and j == 0),
                        stop=(c == NCHUNKS - 1 and j == HCHUNK - 1),
                    )
            wts = workp.tile([128, 128], F32, tag="wts", name=f"wts{ti}")
            nc.scalar.copy(wts[:], wt[:])
            wb = accps.tile([128, 128], F32, name=f"wb{ti}")
            nc.tensor.matmul(wb[:], wts[:], ident[:], is_transpose=True)
            nc.scalar.copy(proj[:, W : 2 * W], wb[:])

        # ---- tail: 8 maps x 2 tensors, h on partitions ----
        tail2d = tailp.tile([128, 2 * TAIL * W], F32)
        tv = tail2d.rearrange("p (m w) -> p m w", w=W)
        nc.sync.dma_start(tv[:, 0:TAIL, :], o_d[MAIN:MAPS].rearrange("m h w -> h m w"))
        nc.sync.dma_start(
            tv[:, TAIL : 2 * TAIL, :], t_d[MAIN:MAPS].rearrange("m h w -> h m w")
        )
        R = tailp.tile([128, 2 * TAIL], F32)
        nc.vector.reduce_sum(R[:], tv, axis=AX.X)
        To = tailp.tile([TAIL, 2 * W], F32)
        Tt = tailp.tile([TAIL, 2 * W], F32)
        # One PSUM bank per tensor's tail: the R-transpose at base partition
        # 0 plus two ones-matmul [1,512] chunks at bases 32/64 (matmul PSUM
        # outputs may only start at partitions 0/32/64).
        tlA = accps.tile([65, 512], F32)
        tlB = accps.tile([65, 512], F32)
        for i, (T, tl) in enumerate(((To, tlA), (Tt, tlB))):
            nc.tensor.matmul(
                tl[0:TAIL, 0:128],
                R[:, i * TAIL : (i + 1) * TAIL],
                ident[:],
                is_transpose=True,
                skip_group_check=True,
            )
            nc.scalar.copy(T[:, 0:W], tl[0:TAIL, 0:128])
            for k in range(2):
                kk = 2 * i + k
                nc.tensor.matmul(
                    tl[32 * (k + 1) : 32 * (k + 1) + 1, :],
                    ones[:],
                    tail2d[:, kk * 512 : (kk + 1) * 512],
                    skip_group_check=True,
                )
        srowA = tailp.tile([65, 512], F32)
        srowB = tailp.tile([65, 512], F32)
        for srow, tl in ((srowA, tlA), (srowB, tlB)):
            nc.scalar.copy(srow[32:33, :], tl[32:33, :])
            nc.scalar.copy(srow[64:65, :], tl[64:65, :])
        nc.gpsimd.dma_start(To[0:4, W : 2 * W], srowA[32:33, :])
        nc.gpsimd.dma_start(To[4:TAIL, W : 2 * W], srowA[64:65, :])
        nc.gpsimd.dma_start(Tt[0:4, W : 2 * W], srowB[32:33, :])
        nc.gpsimd.dma_start(Tt[4:TAIL, W : 2 * W], srowB[64:65, :])

        # ---- softmax + l2 normalize ----
        _normalize(nc, workp, proj_o[:], 128, "po")
        _normalize(nc, workp, proj_t[:], 128, "pt")
        _normalize(nc, workp, To[:], TAIL, "to")
        _normalize(nc, workp, Tt[:], TAIL, "tt")

        # ---- matched dots and per-n channel sums ----
        pos0 = outp.tile([MAIN, 1], F32)
        dumpP = workp.tile([128, 2 * W], F32, tag="dumpP")
        nc.vector.tensor_mul(dumpP[:], proj_o[:], proj_t[:])
        nc.vector.reduce_sum(pos0[:], dumpP[:], axis=AX.X)
        post = outp.tile([TAIL, 1], F32)
        dumpT = workp.tile([TAIL, 2 * W], F32, tag="dumpT")
        nc.vector.tensor_mul(dumpT[:], To[:], Tt[:])
        nc.vector.reduce_sum(post[:], dumpT[:], axis=AX.X)
        nc.tensor.matmul(U[:], g0[:], proj_o[:], start=True, stop=False)
        nc.tensor.matmul(U[:], gt[:], To[:], start=False, stop=True)
        nc.tensor.matmul(Vt[:], g0[:], proj_t[:], start=True, stop=False)
        nc.tensor.matmul(Vt[:], gt[:], Tt[:], start=False, stop=True)
        us = outp.tile([NLOC, 2 * W], F32)
        nc.scalar.copy(us[:], U[:])
        vs = outp.tile([NLOC, 2 * W], F32)
        nc.scalar.copy(vs[:], Vt[:])
        nc.sync.dma_start(u_d, us[:])
        nc.sync.dma_start(v_d, vs[:])
        nc.sync.dma_start(pos_d[0:MAIN, :], pos0[:])
        nc.sync.dma_start(pos_d[MAIN:MAPS, :], post[:])


def _build_nc():
    nc = bacc.Bacc("TRN2", target_bir_lowering=False, debug=False)
    o_d = nc.dram_tensor("o", [MAPS, H, W], F32, kind="ExternalInput").ap()
    t_d = nc.dram_tensor("t", [MAPS, H, W], F32, kind="ExternalInput").ap()
    id_d = nc.dram_tensor("ident", [128, 128], F32, kind="ExternalInput").ap()
    g0_d = nc.dram_tensor("g0", [128, NLOC], F32, kind="ExternalInput").ap()
    gt_d = nc.dram_tensor("gt", [TAIL, NLOC], F32, kind="ExternalInput").ap()
    on_d = nc.dram_tensor("ones", [128, 1], F32, kind="ExternalInput").ap()
    pos_d = nc.dram_tensor("pos", [MAPS, 1], F32, kind="ExternalOutput").ap()
    u_d = nc.dram_tensor("u", [NLOC, 2 * W], F32, kind="ExternalOutput").ap()
    v_d = nc.dram_tensor("v", [NLOC, 2 * W], F32, kind="ExternalOutput").ap()
    with tile.TileContext(nc) as tc:
        _body(tc, o_d, t_d, id_d, g0_d, gt_d, on_d, pos_d, u_d, v_d)
    nc.compile()
    return nc


_NC = None


def _get_nc():
    global _NC
    if _NC is None:
        _NC = _build_nc()
    return _NC


_IDENT = np.eye(128, dtype=np.float32)
_G0 = np.zeros((128, NLOC), np.float32)
_G0[np.arange(128), np.arange(128) // C] = 1.0
_GT = np.zeros((TAIL, NLOC), np.float32)
_GT[:, NLOC - 1] = 1.0
_ONES = np.ones((128, 1), np.float32)


def _make_in_maps(output, target):
    in_maps = []
    for i in range(NCORES):
        o = np.ascontiguousarray(output[i * NLOC : (i + 1) * NLOC]).reshape(MAPS, H, W)
        t = np.ascontiguousarray(target[i * NLOC : (i + 1) * NLOC]).reshape(MAPS, H, W)
        in_maps.append(
            {"o": o, "t": t, "ident": _IDENT, "g0": _G0, "gt": _GT, "ones": _ONES}
        )
    return in_maps


def _finish(results):
    A = 0.0
    B = 0.0
    for r in results:
        A += float(r["pos"].astype(np.float64).sum())
        B += float((r["u"].astype(np.float64) * r["v"].astype(np.float64)).sum())
    # sim_pos = 0.5*A/(N*C); sim = 0.5*B/N; loss = -log(sim_pos/sim)/(C*N)
    loss = -np.log(A / (C * B)) / (C * N)
    return np.float32(loss)


def kernel(output, target):
    output = np.asarray(output, dtype=np.float32)
    target = np.asarray(target, dtype=np.float32)
    nc = _get_nc()
    res = run_bass_kernel_spmd(nc, _make_in_maps(output, target), list(range(NCORES)))
    return _finish(res.results)


def profile(output, target):
    """Run once with NTFF tracing; returns max per-core HW exec time in ns."""
    output = np.asarray(output, dtype=np.float32)
    target = np.asarray(target, dtype=np.float32)
    nc = _get_nc()
    res = run_bass_kernel_spmd(
        nc, _make_in_maps(output, target), list(range(NCORES)), trace=True
    )
    return res.exec_time_ns



# revision 2
# speedup vs baseline: 1.0729x; 1.0729x over previous
"""CstLoss on Trainium2 — self-contained Bass/Tile SPMD kernel (8 NeuronCores).

Reference math (per [N=64, C=17, H=128, W=128] f32 pair output/target):
  h/w marginal means of each map -> softmax over the 128-axis -> l2
  normalize -> sim_pos = mean of matched-channel cosines, sim = sum of
  mean-over-batch all-pairs cosines, loss = -log(sim_pos/sim)/C/N.

Key algebra:
  * softmax denominator cancels under l2 normalization (and the reference's
    1e-8 norm clamp never binds since max(exp) = 1), so each projection only
    needs q = e / ||e||_2 with e = exp((S - max S)/W), S = raw row/col sums.
  * ||e||^2 = sum exp(2z) -- computed with a second Exp pass + accum_out, so
    the ACT engine only ever loads the Exp / Sqrt / Copy tables.
  * sum_ij dot(qo_i, qt_j) = dot(sum_i qo_i, sum_j qt_j): the CxC pair matrix
    is never materialized; per n we only need channel sums U, V. The per-map
    1/||e|| factors are folded into the channel-membership matrices (g0, gt)
    and into the matched-dot scalars, so the big [128, 256] e-tensors are
    never rescaled.
  * everything reduces on-device to 3 scalars per core (A = sum of matched
    cosines split by segment, B = sum_n U.V); the host all-reduces and takes
    the log.

Sharding: data-parallel over batch, 8 entries (136 maps) per core.

Schedule (the point of this rewrite): the two small-descriptor tail DMAs are
issued FIRST (empty ring -> fast descriptor gen, data on-chip by ~12us, all
tail compute hidden), then all 17 chunk DMAs back-to-back on the sync queue
(o-tensor first, so o's softmax/normalize overlaps t's DMA window; t's last
chunk is split small so the final DVE reduce off the critical path is short).
Per chunk: DVE segmented reduce (row sums) + PE transpose-accumulate into
PSUM (col sums). Only t's last-chunk reduce, t's normalize, one Sqrt table
load, the tiny final matmuls and one 16-byte output DMA trail the last byte.
"""

import contextlib
import ctypes
import sys
import types
from contextlib import ExitStack

import numpy as np

import concourse.bacc as bacc
import concourse.tile as tile
from concourse import mybir
from concourse.bass_utils import run_bass_kernel_spmd

F32 = mybir.dt.float32
AX = mybir.AxisListType
ACT = mybir.ActivationFunctionType

N, C, H, W = 64, 17, 128, 128
NCORES = 8
NLOC = N // NCORES           # 8 batch entries per core
MAPS = NLOC * C              # 136 maps per tensor per core
MAIN = 128                   # maps in the main batch
TAIL = MAPS - MAIN           # 8 maps in the tail
ROWS_O = [16] * 8            # h-rows per chunk, tensor o
ROWS_T = [16] * 7 + [8, 8]   # smaller final chunks: short last reduce


def _install_ntff_hook():
    """Provide antenv.axon_hooks if the image lacks it (needed only when
    run_bass_kernel_spmd is called with trace=True; harmless otherwise)."""
    if "antenv.axon_hooks" in sys.modules:
        return
    so_path = "/opt/axon/libaxon_pjrt.so"
    hook = None
    try:
        lib = ctypes.CDLL(so_path)
        if hasattr(lib, "axon_start_nrt_profile"):
            lib.axon_start_nrt_profile.argtypes = [
                ctypes.POINTER(ctypes.c_int64),
                ctypes.c_size_t,
            ]
            lib.axon_start_nrt_profile.restype = ctypes.c_int64
            lib.axon_stop_nrt_profile.argtypes = [ctypes.c_char_p]
            lib.axon_stop_nrt_profile.restype = ctypes.c_int64

            @contextlib.contextmanager
            def _hook(output_dir, device_ids):
                import jax

                jax.devices()
                if device_ids:
                    ids = (ctypes.c_int64 * len(device_ids))(*device_ids)
                    rc = lib.axon_start_nrt_profile(ids, len(device_ids))
                else:
                    rc = lib.axon_start_nrt_profile(None, 0)
                if rc != 0:
                    raise RuntimeError(f"axon_start_nrt_profile rc={rc}")
                try:
                    yield
                finally:
                    n = lib.axon_stop_nrt_profile(str(output_dir).encode())
                    print(f"profile: {n} file(s) in {output_dir}", file=sys.stderr)

            hook = _hook
    except OSError:
        pass
    mod = types.ModuleType("antenv.axon_hooks")
    mod.get_axon_ntff_profile_hook = lambda: hook
    mod.set_axon_ntff_profile_hook = lambda h: None
    sys.modules["antenv.axon_hooks"] = mod


_install_ntff_hook()


def _body(tc, o_d, t_d, id_d, g0_d, gt_d, on_d, res_d):
    nc = tc.nc
    with ExitStack() as ctx:
        consts = ctx.enter_context(tc.tile_pool(name="consts", bufs=1))
        chunks = ctx.enter_context(tc.tile_pool(name="chunks", bufs=1))
        tailp = ctx.enter_context(tc.tile_pool(name="tailp", bufs=1))
        projp = ctx.enter_context(tc.tile_pool(name="projp", bufs=1))
        workp = ctx.enter_context(tc.tile_pool(name="workp", bufs=1))
        outp = ctx.enter_context(tc.tile_pool(name="outp", bufs=1))
        # PSUM: 8 distinct tiles = 8 banks, no slot rotation (slot reuse
        # with concurrent PE traffic wedges the device: NRT status 101).
        accps = ctx.enter_context(tc.tile_pool(name="accps", bufs=1, space="PSUM"))

        # ---- consts on the gpsimd (SWDGE) queue: off the sync FIFO ----
        ident = consts.tile([128, 128], F32)
        nc.gpsimd.dma_start(ident[:], id_d)
        g0 = consts.tile([128, NLOC], F32)
        nc.gpsimd.dma_start(g0[:], g0_d)
        gt = consts.tile([TAIL, NLOC], F32)
        nc.gpsimd.dma_start(gt[:], gt_d)
        ones = consts.tile([128, 1], F32)
        nc.gpsimd.dma_start(ones[:], on_d)

        # ---- tail DMAs FIRST (1024 x 512B descriptors each: issue into an
        # empty ring; data lands early and all tail compute hides) ----
        tail2d = tailp.tile([128, 2 * TAIL * W], F32)
        tv = tail2d.rearrange("p (m w) -> p m w", w=W)
        nc.sync.dma_start(tv[:, 0:TAIL, :], o_d[MAIN:MAPS].rearrange("m h w -> h m w"))
        nc.sync.dma_start(
            tv[:, TAIL : 2 * TAIL, :], t_d[MAIN:MAPS].rearrange("m h w -> h m w")
        )

        # ---- all main-chunk DMAs issued up front, o first then t ----
        chunk_tiles = {0: [], 1: []}
        for ti, (x_d, rows) in enumerate(((o_d, ROWS_O), (t_d, ROWS_T))):
            r0 = 0
            for c, r in enumerate(rows):
                ck = chunks.tile([128, r * W], F32, name=f"chunk{ti}_{c}")
                nc.sync.dma_start(ck[:], x_d[0:MAIN, r0 : r0 + r, :])
                chunk_tiles[ti].append((ck, r0, r))
                r0 += r

        # PSUM tiles (8 banks)
        wt_o = accps.tile([128, 128], F32)
        wt_t = accps.tile([128, 128], F32)
        wb_o = accps.tile([128, 128], F32)
        wb_t = accps.tile([128, 128], F32)
        tlA = accps.tile([65, 512], F32)
        tlB = accps.tile([65, 512], F32)
        # U also hosts the final scalars in cols 256:259
        U = accps.tile([NLOC, 512], F32)
        Vt = accps.tile([NLOC, 512], F32)

        proj_o = projp.tile([128, 2 * W], F32)
        proj_t = projp.tile([128, 2 * W], F32)
        eo = projp.tile([128, 2 * W], F32)
        et = projp.tile([128, 2 * W], F32)
        To = tailp.tile([TAIL, 2 * W], F32)
        Tt = tailp.tile([TAIL, 2 * W], F32)
        To_e = tailp.tile([TAIL, 2 * W], F32)
        Tt_e = tailp.tile([TAIL, 2 * W], F32)
        # ssq/sq/rn: cols 0:2 = o(h,w), 2:4 = t, 4:6 = tail-o, 6:8 = tail-t
        ssq = workp.tile([128, 8], F32)
        sq = workp.tile([128, 8], F32)
        rn = workp.tile([128, 8], F32)
        dump = workp.tile([128, W], F32)

        # ---- tail compute (data on-chip by ~12us) ----
        R = tailp.tile([128, 2 * TAIL], F32)
        nc.vector.reduce_sum(R[:], tv, axis=AX.X)
        srowA = tailp.tile([65, 512], F32)
        srowB = tailp.tile([65, 512], F32)
        for i, (T, tl, srow) in enumerate(((To, tlA, srowA), (Tt, tlB, srowB))):
            # h-marginals: transpose R's 8 columns -> [8, 128]
            nc.tensor.matmul(
                tl[0:TAIL, 0:128],
                R[:, i * TAIL : (i + 1) * TAIL],
                ident[:],
                is_transpose=True,
                skip_group_check=True,
            )
            nc.vector.tensor_copy(T[:, 0:W], tl[0:TAIL, 0:128])
            # w-marginals: ones-matmul column sums, [1, 512] per 4 maps
            for k in range(2):
                kk = 2 * i + k
                nc.tensor.matmul(
                    tl[32 * (k + 1) : 32 * (k + 1) + 1, :],
                    ones[:],
                    tail2d[:, kk * 512 : (kk + 1) * 512],
                    skip_group_check=True,
                )
            # ACT copies: Copy table load sits mid-window, long before Sqrt
            nc.scalar.copy(srow[32:33, :], tl[32:33, :])
            nc.scalar.copy(srow[64:65, :], tl[64:65, :])
        nc.gpsimd.dma_start(To[0:4, W : 2 * W], srowA[32:33, :])
        nc.gpsimd.dma_start(To[4:TAIL, W : 2 * W], srowA[64:65, :])
        nc.gpsimd.dma_start(Tt[0:4, W : 2 * W], srowB[32:33, :])
        nc.gpsimd.dma_start(Tt[4:TAIL, W : 2 * W], srowB[64:65, :])

        # tail softmax-exp (first Exp table load, hidden)
        for i, (T, T_e) in enumerate(((To, To_e), (Tt, Tt_e))):
            tvv = T.rearrange("p (s w) -> p s w", w=W)
            tev = T_e.rearrange("p (s w) -> p s w", w=W)
            mxt = tailp.tile([TAIL, 2], F32, name=f"mxt{i}")
            nc.vector.reduce_max(mxt[:], tvv, axis=AX.X)
            nbt = tailp.tile([TAIL, 2], F32, name=f"nbt{i}")
            nc.vector.tensor_scalar_mul(nbt[:], mxt[:], -1.0 / W)
            nbt2 = tailp.tile([TAIL, 2], F32, name=f"nbt2{i}")
            nc.vector.tensor_scalar_mul(nbt2[:], mxt[:], -2.0 / W)
            dump8 = tailp.tile([TAIL, W], F32, name=f"dump8{i}")
            for s in range(2):
                nc.scalar.activation(
                    tev[:, s, :], tvv[:, s, :], ACT.Exp,
                    bias=nbt[:, s : s + 1], scale=1.0 / W,
                )
                nc.scalar.activation(
                    dump8[:], tvv[:, s, :], ACT.Exp,
                    bias=nbt2[:, s : s + 1], scale=2.0 / W,
                    accum_out=ssq[0:TAIL, 4 + 2 * i + s : 5 + 2 * i + s],
                )

        # ---- main batches ----
        for ti, (proj, e, wt, wb, base) in enumerate(
            ((proj_o, eo, wt_o, wb_o, 0), (proj_t, et, wt_t, wb_t, 2))
        ):
            nch = len(chunk_tiles[ti])
            for c, (ck, r0, r) in enumerate(chunk_tiles[ti]):
                cv = ck.rearrange("p (h w) -> p h w", w=W)
                nc.vector.reduce_sum(proj[:, r0 : r0 + r], cv, axis=AX.X)
                for j in range(r):
                    nc.tensor.matmul(
                        wt[:],
                        cv[:, j, :],
                        ident[:],
                        is_transpose=True,
                        start=(c == 0 and j == 0),
                        stop=(c == nch - 1 and j == r - 1),
                    )
            wts = workp.tile([128, 128], F32, name=f"wts{ti}")
            nc.vector.tensor_copy(wts[:], wt[:])
            nc.tensor.matmul(wb[:], wts[:], ident[:], is_transpose=True)
            nc.vector.tensor_copy(proj[:, W : 2 * W], wb[:])
            # softmax-exp + ||e||^2 via exp(2z) accumulation
            v3 = proj.rearrange("p (s w) -> p s w", w=W)
            e3 = e.rearrange("p (s w) -> p s w", w=W)
            mx = workp.tile([128, 2], F32, name=f"mx{ti}")
            nc.vector.reduce_max(mx[:], v3, axis=AX.X)
            nb = workp.tile([128, 2], F32, name=f"nb{ti}")
            nc.vector.tensor_scalar_mul(nb[:], mx[:], -1.0 / W)
            nb2 = workp.tile([128, 2], F32, name=f"nb2{ti}")
            nc.vector.tensor_scalar_mul(nb2[:], mx[:], -2.0 / W)
            for s in range(2):
                nc.scalar.activation(
                    e3[:, s, :], v3[:, s, :], ACT.Exp,
                    bias=nb[:, s : s + 1], scale=1.0 / W,
                )
                nc.scalar.activation(
                    dump[:], v3[:, s, :], ACT.Exp,
                    bias=nb2[:, s : s + 1], scale=2.0 / W,
                    accum_out=ssq[:, base + s : base + s + 1],
                )

        # ---- matched (unscaled) dots: can run as soon as both e's exist ----
        dumpP = projp.tile([128, 2 * W], F32)
        nc.vector.tensor_mul(dumpP[:], eo[:], et[:])
        pos_seg = outp.tile([128, 2], F32)
        nc.vector.reduce_sum(pos_seg[:], dumpP.rearrange("p (s w) -> p s w", w=W), axis=AX.X)
        dumpT = tailp.tile([TAIL, 2 * W], F32)
        nc.vector.tensor_mul(dumpT[:], To_e[:], Tt_e[:])
        post_seg = outp.tile([TAIL, 2], F32)
        nc.vector.reduce_sum(post_seg[:], dumpT.rearrange("p (s w) -> p s w", w=W), axis=AX.X)

        # ---- 1/||e|| for all eight streams in one Sqrt (single table load
        # in the serial tail) + DVE reciprocal ----
        nc.scalar.activation(sq[:], ssq[:], ACT.Sqrt)
        nc.vector.reciprocal(rn[:], sq[:])

        # scale matched dots by both rn factors
        nc.vector.tensor_mul(pos_seg[:], pos_seg[:], rn[:, 0:2])
        nc.vector.tensor_mul(pos_seg[:], pos_seg[:], rn[:, 2:4])
        nc.vector.tensor_mul(post_seg[:], post_seg[:], rn[0:TAIL, 4:6])
        nc.vector.tensor_mul(post_seg[:], post_seg[:], rn[0:TAIL, 6:8])

        # fold rn into the channel-membership matrices
        gsc = outp.tile([128, 4 * NLOC], F32)   # o_h, o_w, t_h, t_w
        gtc = outp.tile([TAIL, 4 * NLOC], F32)
        for s in range(4):
            nc.vector.tensor_scalar_mul(
                gsc[:, s * NLOC : (s + 1) * NLOC], g0[:], rn[:, s : s + 1]
            )
            nc.vector.tensor_scalar_mul(
                gtc[:, s * NLOC : (s + 1) * NLOC], gt[:], rn[0:TAIL, 4 + s : 5 + s]
            )

        # U/V channel sums: [8, 256] each, main + tail accumulated
        for i, (e, T_e, P) in enumerate(((eo, To_e, U), (et, Tt_e, Vt))):
            for s in range(2):
                g_col = (2 * i + s) * NLOC
                nc.tensor.matmul(
                    P[:, s * W : (s + 1) * W],
                    gsc[:, g_col : g_col + NLOC],
                    e[:, s * W : (s + 1) * W],
                    start=True, stop=False, skip_group_check=True,
                )
                nc.tensor.matmul(
                    P[:, s * W : (s + 1) * W],
                    gtc[:, g_col : g_col + NLOC],
                    T_e[:, s * W : (s + 1) * W],
                    start=False, stop=True, skip_group_check=True,
                )

        # A (per-segment sums of matched cosines) into U[0:2, 256]
        nc.tensor.matmul(
            U[0:2, 256:257], pos_seg[:], ones[:, 0:1],
            start=True, stop=False, skip_group_check=True,
        )
        nc.tensor.matmul(
            U[0:2, 256:257], post_seg[:], ones[0:TAIL, 0:1],
            start=False, stop=True, skip_group_check=True,
        )

        # B = sum_n U.V into U[0:1, 257]
        Us = outp.tile([NLOC, 2 * W], F32)
        nc.vector.tensor_copy(Us[:], U[:, 0 : 2 * W])
        Vs = outp.tile([NLOC, 2 * W], F32)
        nc.vector.tensor_copy(Vs[:], Vt[:, 0 : 2 * W])
        uv = outp.tile([NLOC, 2 * W], F32)
        nc.vector.tensor_mul(uv[:], Us[:], Vs[:])
        uvs = outp.tile([NLOC, 1], F32)
        nc.vector.reduce_sum(uvs[:], uv[:], axis=AX.X)
        nc.tensor.matmul(
            U[0:1, 257:258], uvs[:], ones[0:TAIL, 0:1], skip_group_check=True
        )

        res_s = outp.tile([2, 2], F32)
        nc.vector.tensor_copy(res_s[0:2, 0:1], U[0:2, 256:257])
        nc.vector.tensor_copy(res_s[0:1, 1:2], U[0:1, 257:258])
        nc.sync.dma_start(res_d, res_s[:])


def _build_nc():
    nc = bacc.Bacc("TRN2", target_bir_lowering=False, debug=False)
    o_d = nc.dram_tensor("o", [MAPS, H, W], F32, kind="ExternalInput").ap()
    t_d = nc.dram_tensor("t", [MAPS, H, W], F32, kind="ExternalInput").ap()
    id_d = nc.dram_tensor("ident", [128, 128], F32, kind="ExternalInput").ap()
    g0_d = nc.dram_tensor("g0", [128, NLOC], F32, kind="ExternalInput").ap()
    gt_d = nc.dram_tensor("gt", [TAIL, NLOC], F32, kind="ExternalInput").ap()
    on_d = nc.dram_tensor("ones", [128, 1], F32, kind="ExternalInput").ap()
    res_d = nc.dram_tensor("res", [2, 2], F32, kind="ExternalOutput").ap()
    with tile.TileContext(nc) as tc:
        _body(tc, o_d, t_d, id_d, g0_d, gt_d, on_d, res_d)
    nc.compile()
    return nc


_NC = None


def _get_nc():
    global _NC
    if _NC is None:
        _NC = _build_nc()
    return _NC


_IDENT = np.eye(128, dtype=np.float32)
_G0 = np.zeros((128, NLOC), np.float32)
_G0[np.arange(128), np.arange(128) // C] = 1.0
_GT = np.zeros((TAIL, NLOC), np.float32)
_GT[:, NLOC - 1] = 1.0
_ONES = np.ones((128, 1), np.float32)


def _make_in_maps(output, target):
    in_maps = []
    for i in range(NCORES):
        o = np.ascontiguousarray(output[i * NLOC : (i + 1) * NLOC]).reshape(MAPS, H, W)
        t = np.ascontiguousarray(target[i * NLOC : (i + 1) * NLOC]).reshape(MAPS, H, W)
        in_maps.append(
            {"o": o, "t": t, "ident": _IDENT, "g0": _G0, "gt": _GT, "ones": _ONES}
        )
    return in_maps


def _finish(results):
    A = 0.0
    B = 0.0
    for r in results:
        res = r["res"].astype(np.float64)
        A += res[0, 0] + res[1, 0]
        B += res[0, 1]
    # sim_pos = 0.5*A/(N*C); sim = 0.5*B/N; loss = -log(sim_pos/sim)/(C*N)
    loss = -np.log(A / (C * B)) / (C * N)
    return np.float32(loss)


def kernel(output, target):
    output = np.asarray(output, dtype=np.float32)
    target = np.asarray(target, dtype=np.float32)
    nc = _get_nc()
    res = run_bass_kernel_spmd(nc, _make_in_maps(output, target), list(range(NCORES)))
    return _finish(res.results)


def profile(output, target):
    """Run once with NTFF tracing; returns max per-core HW exec time in ns."""
    output = np.asarray(output, dtype=np.float32)
    target = np.asarray(target, dtype=np.float32)
    nc = _get_nc()
    res = run_bass_kernel_spmd(
        nc, _make_in_maps(output, target), list(range(NCORES)), trace=True
    )
    return res.exec_time_ns


# revision 5
# speedup vs baseline: 1.1102x; 1.0347x over previous
"""CstLoss on Trainium2 — self-contained Bass/Tile SPMD kernel (8 NeuronCores).

Reference math (per [N=64, C=17, H=128, W=128] f32 pair output/target):
  h/w marginal means of each map -> softmax over the 128-axis -> l2
  normalize -> sim_pos = mean of matched-channel cosines, sim = sum of
  mean-over-batch all-pairs cosines, loss = -log(sim_pos/sim)/C/N.

Key algebra:
  * softmax denominator cancels under l2 normalization (and the reference's
    1e-8 norm clamp never binds since max(exp) = 1), so each projection only
    needs q = e / ||e||_2 with e = exp((S - max S)/W), S = raw row/col sums.
  * ||e||^2 = sum exp(2z) (second Exp pass + accum_out) and
    1/||e|| = exp(-0.5 ln ||e||^2), so with Copy the ACT engine only needs
    functions from ONE table set (natural_log_exp_and_others; the greedy
    set selector is steered there by _patch_act_tables) -> a single
    ACT_TABLE_LOAD for the whole kernel, fully hidden under the DMA window.
  * sum_ij dot(qo_i, qt_j) = dot(sum_i qo_i, sum_j qt_j): the CxC pair
    matrix is never materialized. The per-map 1/||e|| factors are folded
    into the channel-membership matrices (g0, gt) and the matched-dot
    scalars, so the big e-tensors are never rescaled; one [128x16x256]
    matmul per tensor yields both segments' channel sums.
  * on-device reduction to 3 scalars per core (A = matched-cosine sums per
    segment, B = sum_n U.V); host all-reduces and takes the log.

Schedule: everything is issued up front; per-engine FIFO order is arranged
by expected data readiness. Sync-queue DMA order: o0, o1, tail-o, tail-t,
o2..o7, t0..t8 (t's last two chunks are 8 rows, shortening the last DVE
reduce). No SWDGE/gpsimd DMAs at all -- const loads and the tail scatter
ride the scalar HWDGE queue -- because SWDGE descriptor-ring traffic slows
SDMA engines 7/15 and every chunk semaphore gates on the slowest engine.
Per chunk: DVE segmented reduce (row sums) + PE transpose-accumulate (col
sums). All PSUM->SBUF copies are ACT Copy (in every table set); o's
normalize overlaps t's DMA window.
"""

import contextlib
import ctypes
import sys
import types
from contextlib import ExitStack

import numpy as np

import concourse.bacc as bacc
import concourse.hw_specs as hw_specs
import concourse.tile as tile
from concourse import mybir
from concourse.bass_utils import run_bass_kernel_spmd

F32 = mybir.dt.float32
AX = mybir.AxisListType
ACT = mybir.ActivationFunctionType

N, C, H, W = 64, 17, 128, 128
NCORES = 8
NLOC = N // NCORES           # 8 batch entries per core
MAPS = NLOC * C              # 136 maps per tensor per core
MAIN = 128                   # maps in the main batch
TAIL = MAPS - MAIN           # 8 maps in the tail
ROWS_O = [16] * 8            # h-rows per chunk, tensor o
ROWS_T = [16] * 7 + [8, 8]   # smaller final chunks: short last reduce


def _patch_act_tables():
    """Steer the greedy ACT table-set selector to the one set that holds
    every function this kernel uses (exp, ln, copy), so only one
    ACT_TABLE_LOAD is ever emitted."""
    orig = hw_specs.get_activation_tables
    strip = {ACT.Exp, ACT.Ln, ACT.Copy, ACT.Identity}

    def patched(arch):
        tabs = orig(arch)
        return {
            name: (fns if name == "natural_log_exp_and_others" else fns - strip)
            for name, fns in tabs.items()
        }

    bacc.get_activation_tables = patched


def _install_ntff_hook():
    """Provide antenv.axon_hooks if the image lacks it (needed only when
    run_bass_kernel_spmd is called with trace=True; harmless otherwise)."""
    if "antenv.axon_hooks" in sys.modules:
        return
    so_path = "/opt/axon/libaxon_pjrt.so"
    hook = None
    try:
        lib = ctypes.CDLL(so_path)
        if hasattr(lib, "axon_start_nrt_profile"):
            lib.axon_start_nrt_profile.argtypes = [
                ctypes.POINTER(ctypes.c_int64),
                ctypes.c_size_t,
            ]
            lib.axon_start_nrt_profile.restype = ctypes.c_int64
            lib.axon_stop_nrt_profile.argtypes = [ctypes.c_char_p]
            lib.axon_stop_nrt_profile.restype = ctypes.c_int64

            @contextlib.contextmanager
            def _hook(output_dir, device_ids):
                import jax

                jax.devices()
                if device_ids:
                    ids = (ctypes.c_int64 * len(device_ids))(*device_ids)
                    rc = lib.axon_start_nrt_profile(ids, len(device_ids))
                else:
                    rc = lib.axon_start_nrt_profile(None, 0)
                if rc != 0:
                    raise RuntimeError(f"axon_start_nrt_profile rc={rc}")
                try:
                    yield
                finally:
                    n = lib.axon_stop_nrt_profile(str(output_dir).encode())
                    print(f"profile: {n} file(s) in {output_dir}", file=sys.stderr)

            hook = _hook
    except OSError:
        pass
    mod = types.ModuleType("antenv.axon_hooks")
    mod.get_axon_ntff_profile_hook = lambda: hook
    mod.set_axon_ntff_profile_hook = lambda h: None
    sys.modules["antenv.axon_hooks"] = mod


_patch_act_tables()
_install_ntff_hook()


def _body(tc, o_d, t_d, id_d, g0_d, gt_d, on_d, res_d):
    nc = tc.nc
    with ExitStack() as ctx:
        consts = ctx.enter_context(tc.tile_pool(name="consts", bufs=1))
        chunks = ctx.enter_context(tc.tile_pool(name="chunks", bufs=1))
        tailp = ctx.enter_context(tc.tile_pool(name="tailp", bufs=1))
        projp = ctx.enter_context(tc.tile_pool(name="projp", bufs=1))
        workp = ctx.enter_context(tc.tile_pool(name="workp", bufs=1))
        outp = ctx.enter_context(tc.tile_pool(name="outp", bufs=1))
        # PSUM: 8 distinct tiles = 8 banks, no slot rotation (slot reuse
        # with concurrent PE traffic wedges the device: NRT status 101).
        accps = ctx.enter_context(tc.tile_pool(name="accps", bufs=1, space="PSUM"))

        # ---- consts on the scalar HWDGE queue (keeps the sync FIFO for
        # bulk data and leaves SWDGE/gpsimd completely unused) ----
        ident = consts.tile([128, 128], F32)
        nc.scalar.dma_start(ident[:], id_d)
        g0 = consts.tile([128, NLOC], F32)
        nc.scalar.dma_start(g0[:], g0_d)
        gt = consts.tile([TAIL, NLOC], F32)
        nc.scalar.dma_start(gt[:], gt_d)
        ones = consts.tile([128, 1], F32)
        nc.scalar.dma_start(ones[:], on_d)

        # ---- sync-queue DMAs: o0, o1, tails, o2.., t.. ----
        chunk_tiles = {0: [], 1: []}
        r0s = {0: 0, 1: 0}

        def issue_chunks(ti, x_d, rows, lo, hi):
            for c in range(lo, hi):
                r = rows[c]
                r0 = r0s[ti]
                ck = chunks.tile([128, r * W], F32, name=f"chunk{ti}_{c}")
                nc.sync.dma_start(ck[:], x_d[0:MAIN, r0 : r0 + r, :])
                chunk_tiles[ti].append((ck, r0, r))
                r0s[ti] += r

        issue_chunks(0, o_d, ROWS_O, 0, 2)
        tail2d = tailp.tile([128, 2 * TAIL * W], F32)
        tv = tail2d.rearrange("p (m w) -> p m w", w=W)
        nc.sync.dma_start(tv[:, 0:TAIL, :], o_d[MAIN:MAPS].rearrange("m h w -> h m w"))
        nc.sync.dma_start(
            tv[:, TAIL : 2 * TAIL, :], t_d[MAIN:MAPS].rearrange("m h w -> h m w")
        )
        issue_chunks(0, o_d, ROWS_O, 2, len(ROWS_O))
        issue_chunks(1, t_d, ROWS_T, 0, len(ROWS_T))

        # PSUM tiles (8 banks)
        wt_o = accps.tile([128, 128], F32)
        wt_t = accps.tile([128, 128], F32)
        wb_o = accps.tile([128, 128], F32)
        wb_t = accps.tile([128, 128], F32)
        tlA = accps.tile([65, 512], F32)
        tlB = accps.tile([65, 512], F32)
        # U8/V8: per-n channel sums, h-segment in cols 0:128, w-segment in
        # cols 128:256. U8 also hosts the final scalars: A segs in
        # [0:2, 256], B in [0:1, 257].
        U8 = accps.tile([NLOC, 512], F32)
        V8 = accps.tile([NLOC, 512], F32)

        proj_o = projp.tile([128, 2 * W], F32)
        proj_t = projp.tile([128, 2 * W], F32)
        eo = projp.tile([128, 2 * W], F32)
        et = projp.tile([128, 2 * W], F32)
        To = tailp.tile([TAIL, 2 * W], F32)
        Tt = tailp.tile([TAIL, 2 * W], F32)
        To_e = tailp.tile([TAIL, 2 * W], F32)
        Tt_e = tailp.tile([TAIL, 2 * W], F32)
        # ssq cols: 0:2 = o(h,w), 2:4 = t, 4:6 = tail-o, 6:8 = tail-t
        ssq = workp.tile([128, 8], F32)
        lssq = workp.tile([128, 8], F32)
        rn = workp.tile([128, 8], F32)
        dump = workp.tile([128, 2 * W], F32)
        dump8 = tailp.tile([TAIL, 2 * W], F32)

        def seg_normalize(P, proj, e, dmp, col, pfx, pool):
            """softmax-exp of one 128-wide segment + ||e||^2 via exp(2z)."""
            mx = pool.tile([P, 1], F32, name=f"mx_{pfx}")
            nc.vector.reduce_max(mx[:], proj, axis=AX.X)
            nb = pool.tile([P, 1], F32, name=f"nb_{pfx}")
            nc.vector.tensor_scalar_mul(nb[:], mx[:], -1.0 / W)
            nb2 = pool.tile([P, 1], F32, name=f"nb2_{pfx}")
            nc.vector.tensor_scalar_mul(nb2[:], mx[:], -2.0 / W)
            nc.scalar.activation(e, proj, ACT.Exp, bias=nb[:], scale=1.0 / W)
            nc.scalar.activation(
                dmp, proj, ACT.Exp, bias=nb2[:], scale=2.0 / W,
                accum_out=ssq[0:P, col : col + 1],
            )

        def do_chunk(ti, proj, wt, c):
            ck, r0, r = chunk_tiles[ti][c]
            cv = ck.rearrange("p (h w) -> p h w", w=W)
            nc.vector.reduce_sum(proj[:, r0 : r0 + r], cv, axis=AX.X)
            nch = len(chunk_tiles[ti])
            for j in range(r):
                nc.tensor.matmul(
                    wt[:],
                    cv[:, j, :],
                    ident[:],
                    is_transpose=True,
                    start=(c == 0 and j == 0),
                    stop=(c == nch - 1 and j == r - 1),
                )

        # ---- o chunks 0-1, then tail compute, then the rest ----
        do_chunk(0, proj_o, wt_o, 0)
        do_chunk(0, proj_o, wt_o, 1)

        # tail: w-marginal ones-matmuls first (only need tail2d bytes)
        for i, tl in enumerate((tlA, tlB)):
            for k in range(2):
                kk = 2 * i + k
                nc.tensor.matmul(
                    tl[32 * (k + 1) : 32 * (k + 1) + 1, :],
                    ones[:],
                    tail2d[:, kk * 512 : (kk + 1) * 512],
                    skip_group_check=True,
                )
        R = tailp.tile([128, 2 * TAIL], F32)
        nc.vector.reduce_sum(R[:], tv, axis=AX.X)
        for i, (T, tl) in enumerate(((To, tlA), (Tt, tlB))):
            nc.tensor.matmul(
                tl[0:TAIL, 0:128],
                R[:, i * TAIL : (i + 1) * TAIL],
                ident[:],
                is_transpose=True,
                skip_group_check=True,
            )
        srowA = tailp.tile([65, 512], F32)
        srowB = tailp.tile([65, 512], F32)
        for i, (T, tl, srow) in enumerate(((To, tlA, srowA), (Tt, tlB, srowB))):
            nc.scalar.copy(srow[32:33, :], tl[32:33, :])
            nc.scalar.copy(srow[64:65, :], tl[64:65, :])
            nc.scalar.copy(T[:, 0:W], tl[0:TAIL, 0:128])
        nc.scalar.dma_start(To[0:4, W : 2 * W], srowA[32:33, :])
        nc.scalar.dma_start(To[4:TAIL, W : 2 * W], srowA[64:65, :])
        nc.scalar.dma_start(Tt[0:4, W : 2 * W], srowB[32:33, :])
        nc.scalar.dma_start(Tt[4:TAIL, W : 2 * W], srowB[64:65, :])
        for i, (T, T_e) in enumerate(((To, To_e), (Tt, Tt_e))):
            for s in range(2):
                seg_normalize(
                    TAIL, T[:, s * W : (s + 1) * W], T_e[:, s * W : (s + 1) * W],
                    dump8[:, s * W : (s + 1) * W], 4 + 2 * i + s, f"t{i}{s}", tailp,
                )

        # ---- main o ----
        for c in range(2, len(chunk_tiles[0])):
            do_chunk(0, proj_o, wt_o, c)
        seg_normalize(128, proj_o[:, 0:W], eo[:, 0:W], dump[:, 0:W], 0, "oh", workp)
        wts_o = workp.tile([128, 128], F32)
        nc.scalar.copy(wts_o[:], wt_o[:])
        nc.tensor.matmul(wb_o[:], wts_o[:], ident[:], is_transpose=True)
        nc.scalar.copy(proj_o[:, W : 2 * W], wb_o[:])
        seg_normalize(
            128, proj_o[:, W : 2 * W], eo[:, W : 2 * W], dump[:, W : 2 * W], 1, "ow", workp
        )

        # ---- main t (tail dots slotted mid-stream) ----
        for c in range(0, 3):
            do_chunk(1, proj_t, wt_t, c)
        post_seg = outp.tile([TAIL, 2], F32)
        nc.vector.tensor_mul(dump8[:], To_e[:], Tt_e[:])
        nc.vector.reduce_sum(
            post_seg[:], dump8.rearrange("p (s w) -> p s w", w=W), axis=AX.X
        )
        for c in range(3, len(chunk_tiles[1])):
            do_chunk(1, proj_t, wt_t, c)
        seg_normalize(128, proj_t[:, 0:W], et[:, 0:W], dump[:, 0:W], 2, "th", workp)
        wts_t = workp.tile([128, 128], F32)
        nc.scalar.copy(wts_t[:], wt_t[:])
        nc.tensor.matmul(wb_t[:], wts_t[:], ident[:], is_transpose=True)
        nc.scalar.copy(proj_t[:, W : 2 * W], wb_t[:])
        seg_normalize(
            128, proj_t[:, W : 2 * W], et[:, W : 2 * W], dump[:, W : 2 * W], 3, "tw", workp
        )

        # ---- matched (unscaled) dots ----
        pos_seg = outp.tile([128, 2], F32)
        nc.vector.tensor_mul(dump[:], eo[:], et[:])
        nc.vector.reduce_sum(
            pos_seg[:], dump.rearrange("p (s w) -> p s w", w=W), axis=AX.X
        )

        # ---- 1/||e|| = exp(-0.5 ln ssq), all 8 streams at once ----
        nc.scalar.activation(lssq[:], ssq[:], ACT.Ln)
        nc.scalar.activation(rn[:], lssq[:], ACT.Exp, scale=-0.5)

        # scale matched dots by both rn factors
        nc.vector.tensor_mul(pos_seg[:], pos_seg[:], rn[:, 0:2])
        nc.vector.tensor_mul(pos_seg[:], pos_seg[:], rn[:, 2:4])
        nc.vector.tensor_mul(post_seg[:], post_seg[:], rn[0:TAIL, 4:6])
        nc.vector.tensor_mul(post_seg[:], post_seg[:], rn[0:TAIL, 6:8])

        # fold rn into the channel-membership matrices: cols 0:8 h, 8:16 w
        gsc_o = outp.tile([128, 2 * NLOC], F32)
        gsc_t = outp.tile([128, 2 * NLOC], F32)
        gtc_o = outp.tile([TAIL, 2 * NLOC], F32)
        gtc_t = outp.tile([TAIL, 2 * NLOC], F32)
        for s in range(2):
            nc.vector.tensor_scalar_mul(
                gsc_o[:, s * NLOC : (s + 1) * NLOC], g0[:], rn[:, s : s + 1]
            )
            nc.vector.tensor_scalar_mul(
                gsc_t[:, s * NLOC : (s + 1) * NLOC], g0[:], rn[:, 2 + s : 3 + s]
            )
            nc.vector.tensor_scalar_mul(
                gtc_o[:, s * NLOC : (s + 1) * NLOC], gt[:], rn[0:TAIL, 4 + s : 5 + s]
            )
            nc.vector.tensor_scalar_mul(
                gtc_t[:, s * NLOC : (s + 1) * NLOC], gt[:], rn[0:TAIL, 6 + s : 7 + s]
            )

        # U/V channel sums: per segment, main + tail accumulated
        for P, gm, gtl, e, T_e in (
            (U8, gsc_o, gtc_o, eo, To_e),
            (V8, gsc_t, gtc_t, et, Tt_e),
        ):
            for s in range(2):
                nc.tensor.matmul(
                    P[:, s * W : (s + 1) * W],
                    gm[:, s * NLOC : (s + 1) * NLOC],
                    e[:, s * W : (s + 1) * W],
                    start=True, stop=False, skip_group_check=True,
                )
                nc.tensor.matmul(
                    P[:, s * W : (s + 1) * W],
                    gtl[:, s * NLOC : (s + 1) * NLOC],
                    T_e[:, s * W : (s + 1) * W],
                    start=False, stop=True, skip_group_check=True,
                )

        # A (per-segment matched-cosine sums) into U8[0:2, 256]
        nc.tensor.matmul(
            U8[0:2, 256:257], pos_seg[:], ones[:, 0:1],
            start=True, stop=False, skip_group_check=True,
        )
        nc.tensor.matmul(
            U8[0:2, 256:257], post_seg[:], ones[0:TAIL, 0:1],
            start=False, stop=True, skip_group_check=True,
        )

        # B = sum_n U.V into U8[0:1, 257] (uv reads U8 straight from PSUM;
        # one full-row reduce covers both segments)
        Vs = outp.tile([NLOC, 2 * W], F32)
        nc.scalar.copy(Vs[:], V8[:, 0 : 2 * W])
        uv = outp.tile([NLOC, 2 * W], F32)
        nc.vector.tensor_mul(uv[:], U8[:, 0 : 2 * W], Vs[:])
        uvs = outp.tile([NLOC, 1], F32)
        nc.vector.reduce_sum(uvs[:], uv[:], axis=AX.X)
        nc.tensor.matmul(
            U8[0:1, 257:258], uvs[:], ones[0:NLOC, 0:1], skip_group_check=True
        )

        res_s = outp.tile([2, 2], F32)
        nc.scalar.copy(res_s[0:2, 0:1], U8[0:2, 256:257])
        nc.scalar.copy(res_s[0:1, 1:2], U8[0:1, 257:258])
        nc.sync.dma_start(res_d, res_s[:])


def _build_nc():
    nc = bacc.Bacc("TRN2", target_bir_lowering=False, debug=False)
    o_d = nc.dram_tensor("o", [MAPS, H, W], F32, kind="ExternalInput").ap()
    t_d = nc.dram_tensor("t", [MAPS, H, W], F32, kind="ExternalInput").ap()
    id_d = nc.dram_tensor("ident", [128, 128], F32, kind="ExternalInput").ap()
    g0_d = nc.dram_tensor("g0", [128, NLOC], F32, kind="ExternalInput").ap()
    gt_d = nc.dram_tensor("gt", [TAIL, NLOC], F32, kind="ExternalInput").ap()
    on_d = nc.dram_tensor("ones", [128, 1], F32, kind="ExternalInput").ap()
    res_d = nc.dram_tensor("res", [2, 2], F32, kind="ExternalOutput").ap()
    with tile.TileContext(nc) as tc:
        _body(tc, o_d, t_d, id_d, g0_d, gt_d, on_d, res_d)
    nc.compile()
    return nc


_NC = None


def _get_nc():
    global _NC
    if _NC is None:
        _NC = _build_nc()
    return _NC


_IDENT = np.eye(128, dtype=np.float32)
_G0 = np.zeros((128, NLOC), np.float32)
_G0[np.arange(128), np.arange(128) // C] = 1.0
_GT = np.zeros((TAIL, NLOC), np.float32)
_GT[:, NLOC - 1] = 1.0
_ONES = np.ones((128, 1), np.float32)


def _make_in_maps(output, target):
    in_maps = []
    for i in range(NCORES):
        o = np.ascontiguousarray(output[i * NLOC : (i + 1) * NLOC]).reshape(MAPS, H, W)
        t = np.ascontiguousarray(target[i * NLOC : (i + 1) * NLOC]).reshape(MAPS, H, W)
        in_maps.append(
            {"o": o, "t": t, "ident": _IDENT, "g0": _G0, "gt": _GT, "ones": _ONES}
        )
    return in_maps


def _finish(results):
    A = 0.0
    B = 0.0
    for r in results:
        res = r["res"].astype(np.float64)
        A += res[0, 0] + res[1, 0]
        B += res[0, 1]
    # sim_pos = 0.5*A/(N*C); sim = 0.5*B/N; loss = -log(sim_pos/sim)/(C*N)
    loss = -np.log(A / (C * B)) / (C * N)
    return np.float32(loss)


def kernel(output, target):
    output = np.asarray(output, dtype=np.float32)
    target = np.asarray(target, dtype=np.float32)
    nc = _get_nc()
    res = run_bass_kernel_spmd(nc, _make_in_maps(output, target), list(range(NCORES)))
    return _finish(res.results)


def profile(output, target):
    """Run once with NTFF tracing; returns max per-core HW exec time in ns."""
    output = np.asarray(output, dtype=np.float32)
    target = np.asarray(target, dtype=np.float32)
    nc = _get_nc()
    res = run_bass_kernel_spmd(
        nc, _make_in_maps(output, target), list(range(NCORES)), trace=True
    )
    return res.exec_time_ns
